# revision 60
# baseline (speedup 1.0000x reference)
"""Trainium2 Bass kernel for the Kruskal (CP/Tucker) linear layer.

Math: the reference reconstructs W (4096x4096) from a rank-16 CP core and
Tucker factors, then computes y = x @ W.T + bias.  Because the 6D core is a
CP (Kruskal) tensor of rank 16, W itself is exactly rank 16:

    W = g_out @ g_in.T
    g_in[def, r]  = (f3@c3)[d,r] * (f4@c4)[e,r] * (f5@c5)[f,r]   (4096 x 16)
    g_out[abc, r] = (f0@c0)[a,r] * (f1@c1)[b,r] * (f2@c2)[c,r]   (4096 x 16)

so  y = (x @ g_in) @ g_out.T + bias.  The device kernel computes the two
x-dependent projections; the tiny factor-only products (g_in/g_out, ~100
KFLOP) are prepared on the host.

Sharding: data-parallel over the batch (4096 rows -> 8 cores x 512). No
collectives.  x is staged on the host into the SBUF-ready transposed
layout (feature-major bf16) so the device does no transpose at all; the
kernel is DMA-bound at ~8 MB of HBM traffic per core and the DMA stream
runs gapless.

Device schedule per core (512 rows, processed as H=4 pipelined 128-row
quarters):
  - the 8 x chunks (gin packed ahead of chunk 0) stream back-to-back on
    the SP HWDGE queue; the two tiny const DMAs (out-side factors+aux,
    bias row) ride the Pool SWDGE queue so they cost nothing on the
    HWDGE issue track; y chunks DMA out as soon as they drain
  - g_out.T is reconstructed on device from the (8,16) factors: 3 fp32
    matmuls make h0T/h1T/h2T, then 32 per-partition-scalar multiplies
    (DVE + Pool, in their idle window) expand the Khatri-Rao product
    with a single final bf16 rounding
  - stage 1 per quarter: fp32 aux K=1 matmul writes the PSUM ones-row,
    then 32 accumulating matmuls tT(16,128) += gin_kt.T @ x_kt
  - tT copy PSUM->SBUF bf16 (rows 0..15 = t, row 16 = ones/bias row)
  - stage 2 per quarter: 16 matmuls y[:, jt*512:] = [t,1].T @
    [g_out.T; bias] in pairs into 2-bank PSUM tiles, each pair drained
    by one PSUM->SBUF bf16 copy alternating DVE/ACT, y DMA per 2048
    columns
Host upcasts the bf16 y to fp32.
"""

import numpy as np
import ml_dtypes

N_CORES = 8
BATCH = 4096
D = 4096          # in/out features (16*16*16)
R = 16            # CP rank
P = 128           # partitions
NB = BATCH // N_CORES   # 512 batch rows per core
KT = D // P             # 32 feature k-tiles
NT = 512                # matmul moving width / PSUM bank (fp32) columns
JT = D // NT            # 8 output column tiles
H = 4                   # batch quarters (software pipeline stages)
NH = NB // H            # 128 rows per quarter
BC = NH // P            # 1 output row chunk of 128 per quarter
XCH = 2                 # x DMA chunks per quarter (16 k-tiles each)
GW = KT * R             # gin columns (512) packed ahead of x chunk 0
AW = R + 1 + NH         # aux columns (145) packed behind gout

_PROGRAM = None


def _build_program():
    import concourse.tile as tile
    from concourse import bacc, mybir

    nc = bacc.Bacc(
        "TRN2",
        target_bir_lowering=False,
        debug=False,
        enable_asserts=False,
        num_devices=N_CORES,
    )
    # xg cols: [gin (512) | quarter 0..3: kt-major x]
    #   gin[p, kt*R + r] = g_in[kt*128 + p, r]
    #   x part: xg[p, GW + h*KT*NH + kt*NH + n] = x[h*NH + n, kt*128 + p]
    xg_d = nc.dram_tensor("xg", (P, GW + KT * NB), mybir.dt.bfloat16, kind="ExternalInput")
    # out-side factors [c0|f0T|c1|f1T|c2|f2T] (each (8,16)) then on row 0
    # the aux block [e16 (17 cols: zeros, col16=1) | ones (NH cols)], all
    # fp32; g_out.T is reconstructed on device (saves its 150KB DMA) and
    # the aux K=1 matmul (which writes the bias ones-row of tT) runs in
    # fp32.  aux lands on partition 0 (a matmul operand cannot start at
    # partition 16).
    cst_d = nc.dram_tensor("cst", (8, 96 + AW), mybir.dt.float32, kind="ExternalInput")
    # bias row, DMA'd straight into g_out.T's partition-16 row
    bias_d = nc.dram_tensor("bias", (1, D), mybir.dt.bfloat16, kind="ExternalInput")
    y_d = nc.dram_tensor("yc", (NB, D), mybir.dt.bfloat16, kind="ExternalOutput")

    KC = KT // XCH  # k-tiles per x chunk

    with tile.TileContext(nc) as tc:
        with (
            tc.tile_pool(name="const", bufs=1) as constp,
            tc.tile_pool(name="xb", bufs=H * XCH) as xbp,
            tc.tile_pool(name="tsb", bufs=2) as tsbp,
            tc.tile_pool(name="ysb", bufs=4) as ysbp,
            # tT accumulators double-buffer; 2 + 3x2 y-pair banks = 8
            tc.tile_pool(name="tpsum", bufs=2, space="PSUM") as tpsump,
            # each y PSUM tile spans 2 banks (2 matmuls drain in 1 copy)
            tc.tile_pool(name="ypsum", bufs=3, space="PSUM") as ypsump,
        ):
            # The two tiny const DMAs ride the Pool SWDGE queue: their
            # descriptor generation runs on the otherwise-idle Pool engine,
            # so they cost nothing on the HWDGE issue track and slot into
            # the DMA stream right after x chunk 0.  The x chunks stream
            # back-to-back on the SP HWDGE queue.
            gout_sb = constp.tile([R + 1, D], mybir.dt.bfloat16)
            cst_sb = constp.tile([8, 96 + AW], mybir.dt.float32)
            aux_sb = cst_sb[0:1, 96 : 96 + AW]
            nc.gpsimd.dma_start(cst_sb[:], cst_d.ap())
            nc.gpsimd.dma_start(gout_sb[R : R + 1, :], bias_d.ap())
            xs = []
            for h in range(H):
                for ch in range(XCH):
                    i = h * XCH + ch
                    lo = GW + (h * KT + ch * KC) * NH
                    if i == 0:
                        xb = xbp.tile([P, GW + KC * NH], mybir.dt.bfloat16)
                        nc.sync.dma_start(xb[:], xg_d.ap()[:, 0 : lo + KC * NH])
                        gin_sb = xb[:, 0:GW]
                        xs.append(xb[:, GW : GW + KC * NH])
                    else:
                        xb = xbp.tile([P, KC * NH], mybir.dt.bfloat16)
                        nc.sync.dma_start(xb[:], xg_d.ap()[:, lo : lo + KC * NH])
                        xs.append(xb[:])

            # reconstruct g_out.T rows 0..15 on device:
            #   hiT = (f_i @ c_i).T via 3 fp32 matmuls (K=8)
            #   M[r, j1*16+j2] = h1T[r,j1]*h2T[r,j2]   (16 per-partition-
            #   scalar multiplies), then
            #   G[r, j0*256+...] = h0T[r,j0]*M          (16 more)
            # all in fp32 with a single final bf16 rounding, so numerics
            # match the host-computed path; the expansion runs on the
            # otherwise-idle DVE/Pool engines before the first y drains
            h_ps = tpsump.tile([R, 48], mybir.dt.float32, tag="tT_ps")
            for i in range(3):
                nc.tensor.matmul(
                    h_ps[:, i * R : (i + 1) * R],
                    lhsT=cst_sb[:, i * 32 : i * 32 + R],
                    rhs=cst_sb[:, i * 32 + R : i * 32 + 2 * R],
                    start=True,
                    stop=True,
                    skip_group_check=True,
                )
            h_sb = tsbp.tile([R, 48], mybir.dt.float32, tag="h_sb")
            nc.vector.tensor_copy(h_sb[:], h_ps[:])
            m_sb = tsbp.tile([R, 256], mybir.dt.float32, tag="m_sb")
            for j1 in range(R):
                nc.vector.tensor_scalar_mul(
                    m_sb[:, j1 * R : (j1 + 1) * R],
                    h_sb[:, 32:48],
                    h_sb[:, R + j1 : R + j1 + 1],
                )
            # G blocks are consumed left-to-right by the stage-2 pairs:
            # fast DVE takes the early blocks, slow Pool the late ones
            for j0 in range(R):
                eng = nc.vector if j0 < 10 else nc.gpsimd
                eng.tensor_scalar_mul(
                    gout_sb[0:R, j0 * 256 : (j0 + 1) * 256],
                    m_sb[:],
                    h_sb[:, j0 : j0 + 1],
                )

            def s1_aux(h):
                # stage-1 accumulator + K=1 matmul writing ones into row 16
                # and zeros into rows 0..15 (start=True); the stage-1
                # matmuls then accumulate into rows 0..15
                tT_ps = tpsump.tile([R + 1, NH], mybir.dt.float32, tag="tT_ps")
                nc.tensor.matmul(
                    tT_ps[:],
                    lhsT=aux_sb[:, 0 : R + 1],
                    rhs=aux_sb[:, R + 1 : AW],
                    start=True,
                    stop=False,
                    skip_group_check=True,
                )
                return tT_ps

            def s1_chunk(h, ch, tT_ps):
                # one x chunk's worth of tT accumulation
                for k in range(KC):
                    kt = ch * KC + k
                    nc.tensor.matmul(
                        tT_ps[0:R, :],
                        lhsT=gin_sb[:, kt * R : (kt + 1) * R],
                        rhs=xs[h * XCH + ch][:, k * NH : (k + 1) * NH],
                        start=False,
                        stop=(kt == KT - 1),
                        skip_group_check=True,
                    )

            def s1_copy(tT_ps):
                # tT rows 0..15 = (x@g_in).T in bf16, row 16 = ones (bias)
                # (on ACT: DVE carries slightly more y-drain load)
                tT_sb = tsbp.tile([R + 1, NH], mybir.dt.bfloat16)
                nc.scalar.copy(tT_sb[:], tT_ps[:])
                return tT_sb

            class YChunk:
                """Stage 2 for one 128-row chunk: pairs of matmuls write a
                2-bank PSUM tile drained by one copy; copies alternate
                DVE/ACT (GPSIMD/Pool cannot access PSUM); each half of the
                chunk DMAs out as soon as its 4 copies land."""

                def __init__(self, h, bc, tT_sb):
                    self.h, self.bc, self.tT_sb = h, bc, tT_sb
                    self.y_sb = ysbp.tile([P, D], mybir.dt.bfloat16)

                def pair(self, jp):
                    y_ps = ypsump.tile([P, 2 * NT], mybir.dt.float32)
                    for sub in range(2):
                        jt = jp * 2 + sub
                        nc.tensor.matmul(
                            y_ps[:, sub * NT : (sub + 1) * NT],
                            lhsT=self.tT_sb[:, self.bc * P : (self.bc + 1) * P],
                            rhs=gout_sb[:, jt * NT : (jt + 1) * NT],
                        )
                    dst = self.y_sb[:, jp * 2 * NT : (jp + 1) * 2 * NT]
                    if jp % 2 == 0:
                        nc.vector.tensor_copy(dst, y_ps[:])
                    else:
                        nc.scalar.copy(dst, y_ps[:])
                    if jp % 2 == 1:
                        # 2 pairs (2048 cols) landed: stream them out.
                        # SP issues these — its HWDGE queue is idle once
                        # the x DMAs are in flight, and ACT must stay
                        # free for PSUM drains.
                        row = self.h * NH + self.bc * P
                        col = (jp - 1) * 2 * NT
                        nc.sync.dma_start(
                            y_d.ap()[row : row + P, col : col + 4 * NT],
                            self.y_sb[:, col : col + 4 * NT],
                        )

            # pipelined emission: each 128-row quarter runs stage 1 as its
            # two x chunks land, then stage 2 + drains + y DMAs overlap the
            # next quarter's input stream; PE load per quarter window fits
            # with room, so serial emission pipelines cleanly
            for h in range(H):
                tT_psq = s1_aux(h)
                for ch in range(XCH):
                    s1_chunk(h, ch, tT_psq)
                tT_q = s1_copy(tT_psq)
                yq = YChunk(h, 0, tT_q)
                for jp in range(JT // 2):
                    yq.pair(jp)

    nc.compile()
    return nc


def _get_program():
    global _PROGRAM
    if _PROGRAM is None:
        _PROGRAM = _build_program()
    return _PROGRAM


def _host_factors(inputs):
    """Build gin (SBUF layout, bf16), the out-side factor pack (fp32) and
    the bias/aux row (bf16); g_out.T itself is reconstructed on device."""
    c = [np.asarray(inputs[f"c{i}"], dtype=np.float64) for i in range(6)]
    f = [np.asarray(inputs[f"f{i}"], dtype=np.float64) for i in range(6)]
    bias = np.asarray(inputs["bias"], dtype=np.float32)
    h = [f[i] @ c[i] for i in range(6)]  # (16,16) each
    g_in = (
        h[3][:, None, None, :] * h[4][None, :, None, :] * h[5][None, None, :, :]
    ).reshape(D, R)
    # gin SBUF layout: gin_l[p, kt*R + r] = g_in[kt*128 + p, r]
    gin_l = np.ascontiguousarray(
        g_in.reshape(KT, P, R).transpose(1, 0, 2).reshape(P, GW)
    ).astype(ml_dtypes.bfloat16)
    # device computes hiT = ci.T @ fi.T via matmul(lhsT=ci, rhs=fi.T)
    cst = np.zeros((8, 96 + AW), dtype=np.float32)
    for i in range(3):
        cst[:, i * 32 : i * 32 + R] = c[i].astype(np.float32)
        cst[:, i * 32 + R : i * 32 + 2 * R] = f[i].T.astype(np.float32)
    cst[0, 96 + R] = 1.0          # aux e16 one-hot
    cst[0, 96 + R + 1 :] = 1.0    # aux ones
    bias_l = bias.astype(ml_dtypes.bfloat16)[None, :]
    return gin_l, cst, bias_l


# test-harness hooks (unused in graded path)
TRACE = False
LAST_RESULTS = None


def kernel(**inputs):
    from concourse.bass_utils import run_bass_kernel_spmd

    global LAST_RESULTS
    x = np.ascontiguousarray(np.asarray(inputs["x"], dtype=np.float32))
    gin_l, cst, bias_l = _host_factors(inputs)
    nc = _get_program()
    xbf = x.astype(ml_dtypes.bfloat16)
    in_maps = []
    for ci in range(N_CORES):
        xc = xbf[ci * NB : (ci + 1) * NB]  # (NB, D)
        # x part: xg[p, GW + h*KT*NH + kt*NH + n] = xc[h*NH + n, kt*128 + p]
        xl = np.ascontiguousarray(
            xc.reshape(H, NH, KT, P).transpose(3, 0, 2, 1)
        ).reshape(P, KT * NB)
        xg = np.concatenate([gin_l, xl], axis=1)
        in_maps.append({"xg": xg, "cst": cst, "bias": bias_l})
    res = run_bass_kernel_spmd(
        nc, in_maps, core_ids=list(range(N_CORES)), trace=TRACE
    )
    LAST_RESULTS = res
    y = np.concatenate([r["yc"] for r in res.results], axis=0)
    return np.ascontiguousarray(y.astype(np.float32))


if __name__ == "__main__":
    # quick smoke test with random data
    rng = np.random.default_rng(0)
    ins = {"x": rng.normal(size=(BATCH, D)).astype(np.float32)}
    for i in range(6):
        ins[f"c{i}"] = (rng.normal(size=(8, 16)) * 0.1).astype(np.float32)
        ins[f"f{i}"] = (rng.normal(size=(16, 8)) * 0.1).astype(np.float32)
    ins["bias"] = np.zeros(D, dtype=np.float32)
    y = kernel(**ins)
    print("y", y.shape, y.dtype)


# revision 61
# speedup vs baseline: 1.0137x; 1.0137x over previous
"""Trainium2 Bass kernel for the Kruskal (CP/Tucker) linear layer.

Math: the reference reconstructs W (4096x4096) from a rank-16 CP core and
Tucker factors, then computes y = x @ W.T + bias.  Because the 6D core is a
CP (Kruskal) tensor of rank 16, W itself is exactly rank 16:

    W = g_out @ g_in.T
    g_in[def, r]  = (f3@c3)[d,r] * (f4@c4)[e,r] * (f5@c5)[f,r]   (4096 x 16)
    g_out[abc, r] = (f0@c0)[a,r] * (f1@c1)[b,r] * (f2@c2)[c,r]   (4096 x 16)

so  y = (x @ g_in) @ g_out.T + bias.  The device kernel computes the two
x-dependent projections; the tiny factor-only products (g_in/g_out, ~100
KFLOP) are prepared on the host.

Sharding: data-parallel over the batch (4096 rows -> 8 cores x 512). No
collectives.  x is staged on the host into the SBUF-ready transposed
layout (feature-major bf16) so the device does no transpose at all; the
kernel is DMA-bound at ~8 MB of HBM traffic per core and the DMA stream
runs gapless.

Device schedule per core (512 rows, processed as H=4 pipelined 128-row
quarters):
  - the 8 x chunks (gin packed ahead of chunk 0) stream back-to-back on
    the SP HWDGE queue; the two tiny const DMAs (out-side factors+aux,
    bias row) ride the Pool SWDGE queue so they cost nothing on the
    HWDGE issue track; y chunks DMA out as soon as they drain
  - g_out.T is reconstructed on device from the (8,16) factors: 3 fp32
    matmuls make h0T/h1T/h2T, then 32 per-partition-scalar multiplies
    (DVE + Pool, in their idle window) expand the Khatri-Rao product
    with a single final bf16 rounding
  - stage 1 per quarter: fp32 aux K=1 matmul writes the PSUM ones-row,
    then 32 accumulating matmuls tT(16,128) += gin_kt.T @ x_kt
  - tT copy PSUM->SBUF bf16 (rows 0..15 = t, row 16 = ones/bias row)
  - stage 2 per quarter: 16 matmuls y[:, jt*512:] = [t,1].T @
    [g_out.T; bias] in pairs into 2-bank PSUM tiles, each pair drained
    by one PSUM->SBUF bf16 copy alternating DVE/ACT, y DMA per 2048
    columns
Host upcasts the bf16 y to fp32.
"""

import numpy as np
import ml_dtypes

N_CORES = 8
BATCH = 4096
D = 4096          # in/out features (16*16*16)
R = 16            # CP rank
P = 128           # partitions
NB = BATCH // N_CORES   # 512 batch rows per core
KT = D // P             # 32 feature k-tiles
NT = 512                # matmul moving width / PSUM bank (fp32) columns
JT = D // NT            # 8 output column tiles
H = 4                   # batch quarters (software pipeline stages)
NH = NB // H            # 128 rows per quarter
BC = NH // P            # 1 output row chunk of 128 per quarter
XCH = 2                 # x DMA chunks per quarter (16 k-tiles each)
GW = KT * R             # gin columns (512) packed ahead of x chunk 0
AW = R + 1 + NH         # aux columns (145) packed behind gout

_PROGRAM = None


def _build_program():
    import concourse.tile as tile
    from concourse import bacc, mybir

    nc = bacc.Bacc(
        "TRN2",
        target_bir_lowering=False,
        debug=False,
        enable_asserts=False,
        num_devices=N_CORES,
    )
    # Drop the constructor's four const-scalar memsets (const-0.0/1.0/...):
    # nothing in this kernel reads them (scalar.copy passes its bias as an
    # immediate), and they delay Pool's arrival at the entry barrier by
    # ~0.4us, which gates the first x DMA issue.
    blk0 = nc.m.functions[0].blocks[0]
    blk0.instructions = [
        ins for ins in blk0.instructions if not isinstance(ins, mybir.InstMemset)
    ]
    # xg cols: [gin (512) | quarter 0..3: kt-major x]
    #   gin[p, kt*R + r] = g_in[kt*128 + p, r]
    #   x part: xg[p, GW + h*KT*NH + kt*NH + n] = x[h*NH + n, kt*128 + p]
    xg_d = nc.dram_tensor("xg", (P, GW + KT * NB), mybir.dt.bfloat16, kind="ExternalInput")
    # out-side factors [c0|f0T|c1|f1T|c2|f2T] (each (8,16)) then on row 0
    # the aux block [e16 (17 cols: zeros, col16=1) | ones (NH cols)], all
    # fp32; g_out.T is reconstructed on device (saves its 150KB DMA) and
    # the aux K=1 matmul (which writes the bias ones-row of tT) runs in
    # fp32.  aux lands on partition 0 (a matmul operand cannot start at
    # partition 16).
    cst_d = nc.dram_tensor("cst", (8, 96 + AW), mybir.dt.float32, kind="ExternalInput")
    # bias row, DMA'd straight into g_out.T's partition-16 row
    bias_d = nc.dram_tensor("bias", (1, D), mybir.dt.bfloat16, kind="ExternalInput")
    y_d = nc.dram_tensor("yc", (NB, D), mybir.dt.bfloat16, kind="ExternalOutput")

    KC = KT // XCH  # k-tiles per x chunk

    with tile.TileContext(nc) as tc:
        with (
            tc.tile_pool(name="const", bufs=1) as constp,
            tc.tile_pool(name="xb", bufs=H * XCH) as xbp,
            tc.tile_pool(name="tsb", bufs=2) as tsbp,
            tc.tile_pool(name="ysb", bufs=4) as ysbp,
            # tT accumulators double-buffer; 2 + 3x2 y-pair banks = 8
            tc.tile_pool(name="tpsum", bufs=2, space="PSUM") as tpsump,
            # each y PSUM tile spans 2 banks (2 matmuls drain in 1 copy)
            tc.tile_pool(name="ypsum", bufs=3, space="PSUM") as ypsump,
        ):
            # The two tiny const DMAs ride the Pool SWDGE queue: their
            # descriptor generation runs on the otherwise-idle Pool engine,
            # so they cost nothing on the HWDGE issue track and slot into
            # the DMA stream right after x chunk 0.  The x chunks stream
            # back-to-back on the SP HWDGE queue.
            gout_sb = constp.tile([R + 1, D], mybir.dt.bfloat16)
            cst_sb = constp.tile([8, 96 + AW], mybir.dt.float32)
            aux_sb = cst_sb[0:1, 96 : 96 + AW]
            nc.gpsimd.dma_start(cst_sb[:], cst_d.ap())
            nc.gpsimd.dma_start(gout_sb[R : R + 1, :], bias_d.ap())
            xs = []
            for h in range(H):
                for ch in range(XCH):
                    i = h * XCH + ch
                    lo = GW + (h * KT + ch * KC) * NH
                    if i == 0:
                        xb = xbp.tile([P, GW + KC * NH], mybir.dt.bfloat16)
                        nc.sync.dma_start(xb[:], xg_d.ap()[:, 0 : lo + KC * NH])
                        gin_sb = xb[:, 0:GW]
                        xs.append(xb[:, GW : GW + KC * NH])
                    else:
                        xb = xbp.tile([P, KC * NH], mybir.dt.bfloat16)
                        nc.sync.dma_start(xb[:], xg_d.ap()[:, lo : lo + KC * NH])
                        xs.append(xb[:])

            # reconstruct g_out.T rows 0..15 on device:
            #   hiT = (f_i @ c_i).T via 3 fp32 matmuls (K=8)
            #   M[r, j1*16+j2] = h1T[r,j1]*h2T[r,j2]   (16 per-partition-
            #   scalar multiplies), then
            #   G[r, j0*256+...] = h0T[r,j0]*M          (16 more)
            # all in fp32 with a single final bf16 rounding, so numerics
            # match the host-computed path; the expansion runs on the
            # otherwise-idle DVE/Pool engines before the first y drains
            h_ps = tpsump.tile([R, 48], mybir.dt.float32, tag="tT_ps")
            for i in range(3):
                nc.tensor.matmul(
                    h_ps[:, i * R : (i + 1) * R],
                    lhsT=cst_sb[:, i * 32 : i * 32 + R],
                    rhs=cst_sb[:, i * 32 + R : i * 32 + 2 * R],
                    start=True,
                    stop=True,
                    skip_group_check=True,
                )
            h_sb = tsbp.tile([R, 48], mybir.dt.float32, tag="h_sb")
            nc.vector.tensor_copy(h_sb[:], h_ps[:])
            m_sb = tsbp.tile([R, 256], mybir.dt.float32, tag="m_sb")
            for j1 in range(R):
                nc.vector.tensor_scalar_mul(
                    m_sb[:, j1 * R : (j1 + 1) * R],
                    h_sb[:, 32:48],
                    h_sb[:, R + j1 : R + j1 + 1],
                )
            # G blocks are consumed left-to-right by the stage-2 pairs:
            # fast DVE takes the early blocks, slow Pool the late ones
            for j0 in range(R):
                eng = nc.vector if j0 < 10 else nc.gpsimd
                eng.tensor_scalar_mul(
                    gout_sb[0:R, j0 * 256 : (j0 + 1) * 256],
                    m_sb[:],
                    h_sb[:, j0 : j0 + 1],
                )

            def s1_aux(h):
                # stage-1 accumulator + K=1 matmul writing ones into row 16
                # and zeros into rows 0..15 (start=True); the stage-1
                # matmuls then accumulate into rows 0..15
                tT_ps = tpsump.tile([R + 1, NH], mybir.dt.float32, tag="tT_ps")
                nc.tensor.matmul(
                    tT_ps[:],
                    lhsT=aux_sb[:, 0 : R + 1],
                    rhs=aux_sb[:, R + 1 : AW],
                    start=True,
                    stop=False,
                    skip_group_check=True,
                )
                return tT_ps

            def s1_chunk(h, ch, tT_ps):
                # one x chunk's worth of tT accumulation
                for k in range(KC):
                    kt = ch * KC + k
                    nc.tensor.matmul(
                        tT_ps[0:R, :],
                        lhsT=gin_sb[:, kt * R : (kt + 1) * R],
                        rhs=xs[h * XCH + ch][:, k * NH : (k + 1) * NH],
                        start=False,
                        stop=(kt == KT - 1),
                        skip_group_check=True,
                    )

            def s1_copy(tT_ps):
                # tT rows 0..15 = (x@g_in).T in bf16, row 16 = ones (bias)
                # (on ACT: DVE carries slightly more y-drain load)
                tT_sb = tsbp.tile([R + 1, NH], mybir.dt.bfloat16)
                nc.scalar.copy(tT_sb[:], tT_ps[:])
                return tT_sb

            class YChunk:
                """Stage 2 for one 128-row chunk: pairs of matmuls write a
                2-bank PSUM tile drained by one copy; copies alternate
                DVE/ACT (GPSIMD/Pool cannot access PSUM); each half of the
                chunk DMAs out as soon as its 4 copies land."""

                def __init__(self, h, bc, tT_sb):
                    self.h, self.bc, self.tT_sb = h, bc, tT_sb
                    self.y_sb = ysbp.tile([P, D], mybir.dt.bfloat16)

                def pair(self, jp):
                    y_ps = ypsump.tile([P, 2 * NT], mybir.dt.float32)
                    for sub in range(2):
                        jt = jp * 2 + sub
                        nc.tensor.matmul(
                            y_ps[:, sub * NT : (sub + 1) * NT],
                            lhsT=self.tT_sb[:, self.bc * P : (self.bc + 1) * P],
                            rhs=gout_sb[:, jt * NT : (jt + 1) * NT],
                        )
                    dst = self.y_sb[:, jp * 2 * NT : (jp + 1) * 2 * NT]
                    if jp % 2 == 0:
                        nc.vector.tensor_copy(dst, y_ps[:])
                    else:
                        nc.scalar.copy(dst, y_ps[:])
                    if jp % 2 == 1:
                        # 2 pairs (2048 cols) landed: stream them out.
                        # SP issues these — its HWDGE queue is idle once
                        # the x DMAs are in flight, and ACT must stay
                        # free for PSUM drains.
                        row = self.h * NH + self.bc * P
                        col = (jp - 1) * 2 * NT
                        nc.sync.dma_start(
                            y_d.ap()[row : row + P, col : col + 4 * NT],
                            self.y_sb[:, col : col + 4 * NT],
                        )

            # pipelined emission: each 128-row quarter runs stage 1 as its
            # two x chunks land, then stage 2 + drains + y DMAs overlap the
            # next quarter's input stream; PE load per quarter window fits
            # with room, so serial emission pipelines cleanly
            for h in range(H):
                tT_psq = s1_aux(h)
                for ch in range(XCH):
                    s1_chunk(h, ch, tT_psq)
                tT_q = s1_copy(tT_psq)
                yq = YChunk(h, 0, tT_q)
                for jp in range(JT // 2):
                    yq.pair(jp)

    nc.compile()
    return nc


def _get_program():
    global _PROGRAM
    if _PROGRAM is None:
        _PROGRAM = _build_program()
    return _PROGRAM


def _host_factors(inputs):
    """Build gin (SBUF layout, bf16), the out-side factor pack (fp32) and
    the bias/aux row (bf16); g_out.T itself is reconstructed on device."""
    c = [np.asarray(inputs[f"c{i}"], dtype=np.float64) for i in range(6)]
    f = [np.asarray(inputs[f"f{i}"], dtype=np.float64) for i in range(6)]
    bias = np.asarray(inputs["bias"], dtype=np.float32)
    h = [f[i] @ c[i] for i in range(6)]  # (16,16) each
    g_in = (
        h[3][:, None, None, :] * h[4][None, :, None, :] * h[5][None, None, :, :]
    ).reshape(D, R)
    # gin SBUF layout: gin_l[p, kt*R + r] = g_in[kt*128 + p, r]
    gin_l = np.ascontiguousarray(
        g_in.reshape(KT, P, R).transpose(1, 0, 2).reshape(P, GW)
    ).astype(ml_dtypes.bfloat16)
    # device computes hiT = ci.T @ fi.T via matmul(lhsT=ci, rhs=fi.T)
    cst = np.zeros((8, 96 + AW), dtype=np.float32)
    for i in range(3):
        cst[:, i * 32 : i * 32 + R] = c[i].astype(np.float32)
        cst[:, i * 32 + R : i * 32 + 2 * R] = f[i].T.astype(np.float32)
    cst[0, 96 + R] = 1.0          # aux e16 one-hot
    cst[0, 96 + R + 1 :] = 1.0    # aux ones
    bias_l = bias.astype(ml_dtypes.bfloat16)[None, :]
    return gin_l, cst, bias_l


# test-harness hooks (unused in graded path)
TRACE = False
LAST_RESULTS = None


def kernel(**inputs):
    from concourse.bass_utils import run_bass_kernel_spmd

    global LAST_RESULTS
    x = np.ascontiguousarray(np.asarray(inputs["x"], dtype=np.float32))
    gin_l, cst, bias_l = _host_factors(inputs)
    nc = _get_program()
    xbf = x.astype(ml_dtypes.bfloat16)
    in_maps = []
    for ci in range(N_CORES):
        xc = xbf[ci * NB : (ci + 1) * NB]  # (NB, D)
        # x part: xg[p, GW + h*KT*NH + kt*NH + n] = xc[h*NH + n, kt*128 + p]
        xl = np.ascontiguousarray(
            xc.reshape(H, NH, KT, P).transpose(3, 0, 2, 1)
        ).reshape(P, KT * NB)
        xg = np.concatenate([gin_l, xl], axis=1)
        in_maps.append({"xg": xg, "cst": cst, "bias": bias_l})
    res = run_bass_kernel_spmd(
        nc, in_maps, core_ids=list(range(N_CORES)), trace=TRACE
    )
    LAST_RESULTS = res
    y = np.concatenate([r["yc"] for r in res.results], axis=0)
    return np.ascontiguousarray(y.astype(np.float32))


if __name__ == "__main__":
    # quick smoke test with random data
    rng = np.random.default_rng(0)
    ins = {"x": rng.normal(size=(BATCH, D)).astype(np.float32)}
    for i in range(6):
        ins[f"c{i}"] = (rng.normal(size=(8, 16)) * 0.1).astype(np.float32)
        ins[f"f{i}"] = (rng.normal(size=(16, 8)) * 0.1).astype(np.float32)
    ins["bias"] = np.zeros(D, dtype=np.float32)
    y = kernel(**ins)
    print("y", y.shape, y.dtype)
